# revision 22
# baseline (speedup 1.0000x reference)
"""Trainium2 Bass kernel for nn_MetricLoss (pairwise-distance metric loss).

Computation (reference):
    f = x.reshape(1024, 49152)
    G = f @ f.T                      (103 GFLOP Gram matrix)
    dist = sq_i + sq_j - 2 G         (the relu(dist) only binds on the
                                      diagonal, which both masks zero out)
    loss_homo  = 0.5 * sum(same-group dist)
    loss_heter = sum(cross-group relu(1 - dist))

Distribution (8 NeuronCores, one TRN2 chip):
    K-parallel with symmetry: core c holds f[:, c*6144:(c+1)*6144].T as a
    [48, 128, 1024] bf16 tensor (k-major tiles, fully SBUF-resident). Since
    the Gram matrix and both losses are symmetric, each core computes only
    the upper-triangle 128x128 blocks of its partial Gram (36/64 of the
    matmul work) via PSUM-accumulated chains over 4 column chunks, and the
    cross-block loss terms are counted once with weight 2 via a per-core
    weighted mask — the lower triangle is never materialized. Chunked bf16
    ReduceScatters give core c the upper-triangle part of full-K Gram rows
    [128c:128c+128]. Row norms sq are computed in fp32 on the otherwise-idle
    Scalar + Vector + GpSimd engines and summed across cores with a tiny
    fp32 AllReduce issued first (it also absorbs the collective-engine cold
    start). A fused DVE epilogue computes the masked hinge sums per chunk as
    each ReduceScatter lands; the host sums 8x[128,2] partials and
    normalizes.
"""

import numpy as np
import ml_dtypes

import concourse.bass as bass
import concourse.bacc as bacc
import concourse.tile as tile
import concourse.mybir as mybir
import concourse.bass_isa as bass_isa
from concourse.tile_rust import add_dep_helper
from concourse import bass_utils

F32 = mybir.dt.float32
BF16 = mybir.dt.bfloat16
ALU = mybir.AluOpType
AF = mybir.ActivationFunctionType

N_CORES = 8
N = 1024            # batch (rows of f)
K = 64 * 768        # 49152 features per sample
KC = K // N_CORES   # 6144 features per core
KT = KC // 128      # 48 k-tiles of 128 per core
BK = 8              # samples per class group
MB = N // 128       # 8 row blocks

CWS = [384, 256, 256, 128]          # column-chunk widths
OFF = [0, 384, 640, 896]            # chunk column offsets
NJ = len(CWS)
# process biggest chunks first (hides the input DMA), smallest last
CH_ORDER = [2, 1, 0, 3]

_CACHE = {}


def _chunk_chains(j):
    """Upper-triangle chains for chunk j: (m, col_off, width)."""
    chains = []
    for m in range(MB):
        lo = max(OFF[j], 128 * m)
        hi = OFF[j] + CWS[j]
        if lo < hi:
            chains.append((m, lo, hi - lo))
    return chains


def _build_nc():
    nc = bacc.Bacc("TRN2", target_bir_lowering=False, debug=False,
                   num_devices=N_CORES)

    ft = nc.dram_tensor("ft", [KT, 128, N], BF16, kind="ExternalInput").ap()
    mask_same = nc.dram_tensor("mask_same", [128, N], F32,
                               kind="ExternalInput").ap()
    # weighted cross-group mask: 0 below own block, 1 within own block,
    # 2 above it (upper-triangle pair counting)
    mask_diff = nc.dram_tensor("mask_diff", [128, N], F32,
                               kind="ExternalInput").ap()
    emask = nc.dram_tensor("emask", [128, 8], F32, kind="ExternalInput").ap()
    out = nc.dram_tensor("out", [128, 2], F32, kind="ExternalOutput").ap()

    rg = [list(range(N_CORES))]

    with tile.TileContext(nc) as tc:
        with (
            tc.tile_pool(name="ftp", bufs=1) as ftp,
            tc.tile_pool(name="misc", bufs=1) as misc,
            tc.tile_pool(name="gcopy", bufs=4) as gcp,
            tc.tile_pool(name="sqt", bufs=3) as sqtp,
            tc.tile_pool(name="junk", bufs=1) as jkp,
            tc.tile_pool(name="psum", bufs=8, space="PSUM") as psp,
            tc.tile_pool(name="dram", bufs=1, space="DRAM") as drp,
        ):
            # ---- load inputs to SBUF ----
            ft_sb = []
            for k in range(KT):
                t = ftp.tile([128, N], BF16, tag=f"ft{k}", name=f"ft{k}")
                nc.sync.dma_start(t[:], ft[k])
                ft_sb.append(t)

            ms_sb = misc.tile([128, N], F32, tag="ms", name="ms")
            md_sb = misc.tile([128, N], F32, tag="md", name="md")
            em_sb = misc.tile([128, 8], F32, tag="em", name="em")
            nc.sync.dma_start(ms_sb[:], mask_same[:])
            nc.sync.dma_start(md_sb[:], mask_diff[:])
            nc.sync.dma_start(em_sb[:], emask[:])

            # ---- sq pipeline on ACT (square) + DVE (accumulate) ----
            acc = misc.tile([128, N], F32, tag="acc", name="acc")
            nc.vector.memset(acc[:], 0.0)
            for k in range(KT):
                sqt = sqtp.tile([128, N], F32, tag="sqt", name=f"sqt{k}")
                if k % 2 == 0:
                    nc.scalar.activation(sqt[:], ft_sb[k][:], AF.Square)
                else:
                    nc.vector.tensor_tensor(sqt[:], ft_sb[k][:], ft_sb[k][:],
                                            ALU.mult)
                nc.vector.tensor_tensor(acc[:], acc[:], sqt[:], ALU.add)

            sqb = drp.tile([1, N], F32, tag="sqb", name="sqb")
            sq_ar = drp.tile([1, N], F32, tag="sqar", name="sq_ar")
            # cross-partition reduce of acc without touching the gpsimd
            # queue: error-free bf16 hi/lo split, 16 bf16 DMA transposes,
            # fp32 DVE reduces
            acc_hi = misc.tile([128, N], BF16, tag="ahi", name="acc_hi")
            acc_lo = misc.tile([128, N], BF16, tag="alo", name="acc_lo")
            lo_f = misc.tile([128, N], F32, tag="alof", name="lo_f")
            nc.vector.tensor_copy(acc_hi[:], acc[:])
            nc.vector.tensor_tensor(lo_f[:], acc[:], acc_hi[:], ALU.subtract)
            nc.vector.tensor_copy(acc_lo[:], lo_f[:])
            hiT = misc.tile([128, N], BF16, tag="hiT", name="hiT")
            loT = misc.tile([128, N], BF16, tag="loT", name="loT")
            sq_hi = misc.tile([128, 8], F32, tag="sqh", name="sq_hi")
            sq_lo = misc.tile([128, 8], F32, tag="sql", name="sq_lo")
            sqc = misc.tile([128, 8], F32, tag="sqc", name="sqc")
            for cix in range(MB):
                tsl = slice(cix * 128, (cix + 1) * 128)
                nc.sync.dma_start(hiT[:, tsl], acc_hi[:, tsl], transpose=True)
                nc.sync.dma_start(loT[:, tsl], acc_lo[:, tsl], transpose=True)
                nc.vector.reduce_sum(sq_hi[:, cix:cix + 1], hiT[:, tsl],
                                     axis=mybir.AxisListType.X)
                nc.vector.reduce_sum(sq_lo[:, cix:cix + 1], loT[:, tsl],
                                     axis=mybir.AxisListType.X)
                nc.vector.tensor_tensor(sqc[:, cix:cix + 1],
                                        sq_hi[:, cix:cix + 1],
                                        sq_lo[:, cix:cix + 1], ALU.add)
                nc.sync.dma_start(sqb[0:1, cix * 128:(cix + 1) * 128],
                                  sqc[:, cix:cix + 1])
            # warmup collective: rings the CC doorbell at t~5us so the
            # collective engine's cold start overlaps the compute
            warm_sb = misc.tile([1, 8], F32, tag="wsb", name="warm_sb")
            nc.vector.memset(warm_sb[:], 0.0)
            warm_in = drp.tile([1, 8], F32, tag="wi", name="warm_in")
            warm_out = drp.tile([1, 8], F32, tag="wo", name="warm_out")
            nc.sync.dma_start(warm_in[:], warm_sb[:])
            warm_cc = nc.gpsimd.collective_compute(
                "AllReduce", ALU.add, replica_groups=rg,
                ins=[warm_in.opt()], outs=[warm_out.opt()])
            ar_cc = nc.gpsimd.collective_compute(
                "AllReduce", ALU.add, replica_groups=rg,
                ins=[sqb.opt()], outs=[sq_ar.opt()])
            add_dep_helper(ar_cc.ins, warm_cc.ins, False,
                           "pin collective-queue order")
            prev_cc = ar_cc

            bounce = {}
            rs = {}
            for j in range(NJ):
                bounce[j] = drp.tile([N, CWS[j]], BF16, tag=f"bnc{j}",
                                     name=f"bnc{j}")
                rs[j] = drp.tile([128, CWS[j]], BF16, tag=f"rs{j}",
                                 name=f"rs{j}")

            # zero the never-written (lower-triangle) bounce regions so the
            # ReduceScatter output is finite everywhere
            zero_sb = misc.tile([128, 384], BF16, tag="z", name="zero_sb")
            nc.vector.memset(zero_sb[:], 0.0)
            for j in range(NJ):
                nb = (OFF[j] + CWS[j]) // 128
                for m in range(nb, MB):
                    nc.sync.dma_start(
                        bounce[j][m * 128:(m + 1) * 128, :],
                        zero_sb[:, 0:CWS[j]])
                for (m, lo, w) in _chunk_chains(j):
                    if lo > OFF[j]:
                        nc.sync.dma_start(
                            bounce[j][m * 128:(m + 1) * 128, 0:lo - OFF[j]],
                            zero_sb[:, 0:lo - OFF[j]])

            # ---- upper-triangle partial Gram ----
            for j in CH_ORDER:
                chs = _chunk_chains(j)
                ptiles = {}
                for (m, lo, w) in chs:
                    ptiles[m] = psp.tile([128, w], F32, tag="chain",
                                         name=f"ch{j}_{m}")
                for k in range(KT):
                    for (m, lo, w) in chs:
                        nc.tensor.matmul(
                            ptiles[m][:],
                            lhsT=ft_sb[k][:, m * 128:(m + 1) * 128],
                            rhs=ft_sb[k][:, lo:lo + w],
                            start=(k == 0),
                            stop=(k == KT - 1),
                        )
                for (m, lo, w) in chs:
                    g = gcp.tile([128, w], BF16, tag="g", name=f"g{j}_{m}")
                    nc.vector.tensor_copy(g[:], ptiles[m][:])
                    nc.sync.dma_start(
                        bounce[j][m * 128:(m + 1) * 128,
                                  lo - OFF[j]:lo - OFF[j] + w], g[:])
                rs_cc = nc.gpsimd.collective_compute(
                    "ReduceScatter", ALU.add, replica_groups=rg,
                    ins=[bounce[j].opt()], outs=[rs[j].opt()])
                add_dep_helper(rs_cc.ins, prev_cc.ins, False,
                               "pin collective-queue order")
                prev_cc = rs_cc

            # ---- sq_row + sq_col broadcast ----
            flat_sb = misc.tile([1, N], F32, tag="flat", name="flat")
            nc.sync.dma_start(flat_sb[:], sq_ar[:])
            S_all = misc.tile([128, 8], F32, tag="S", name="S")
            for b in range(MB):
                nc.sync.dma_start(S_all[:, b:b + 1],
                                  sq_ar[0:1, b * 128:(b + 1) * 128])
            sq_row = misc.tile([128, 1], F32, tag="sqr", name="sqr")
            junk8 = misc.tile([128, 8], F32, tag="jk8", name="junk8")
            nc.vector.tensor_tensor(junk8[:], S_all[:], em_sb[:], ALU.mult)
            nc.vector.reduce_sum(sq_row[:], junk8[:],
                                 axis=mybir.AxisListType.X)
            ones = misc.tile([1, 128], F32, tag="ones", name="ones")
            nc.vector.memset(ones[:], 1.0)
            # PE is idle after the passes; PSUM chain slots are free
            Bs = []
            for h in range(2):
                Bh = psp.tile([128, 512], F32, tag="chain", name=f"B{h}")
                nc.tensor.matmul(Bh[:], lhsT=ones[:],
                                 rhs=flat_sb[0:1, h * 512:(h + 1) * 512],
                                 start=True, stop=True)
                Bs.append(Bh)

            # ---- per-chunk epilogue (fires as each RS lands) ----
            G_sb = misc.tile([128, N], BF16, tag="G", name="G")
            acc_h = []
            acc_e = []
            for j in CH_ORDER:
                W = CWS[j]
                nc.sync.dma_start(G_sb[:, OFF[j]:OFF[j] + W], rs[j][:, :])
                # split at the 512 boundary so each piece maps into one
                # B half
                pieces = []
                lo = OFF[j]
                hi = OFF[j] + W
                while lo < hi:
                    p_hi = min(hi, (lo // 512 + 1) * 512)
                    pieces.append((lo, p_hi - lo))
                    lo = p_hi
                for (plo, pw) in pieces:
                    sl = slice(plo, plo + pw)
                    t0 = jkp.tile([128, pw], F32, tag=f"t0_{plo}",
                                  name=f"t0_{plo}")
                    nc.vector.tensor_scalar_mul(t0[:], G_sb[:, sl], -2.0)
                    d = jkp.tile([128, pw], F32, tag=f"d{plo}",
                                 name=f"d{plo}")
                    Bsl = Bs[plo // 512][:, plo % 512:plo % 512 + pw]
                    nc.vector.scalar_tensor_tensor(
                        out=d[:], in0=t0[:], scalar=sq_row[:], in1=Bsl,
                        op0=ALU.add, op1=ALU.add)
                    ah = misc.tile([128, 1], F32, tag=f"ah{plo}",
                                   name=f"ah{plo}")
                    jh = jkp.tile([128, pw], F32, tag=f"jh{plo}",
                                  name=f"jh{plo}")
                    nc.vector.tensor_tensor(jh[:], d[:], ms_sb[:, sl],
                                            ALU.mult)
                    nc.vector.reduce_sum(ah[:], jh[:],
                                         axis=mybir.AxisListType.X)
                    acc_h.append(ah)
                    # min(d-1, 0) = -relu(1-d); heter partial =
                    # -sum(mask * that), negation applied on the host
                    t1 = jkp.tile([128, pw], F32, tag=f"t1_{plo}",
                                  name=f"t1_{plo}")
                    nc.vector.tensor_scalar(
                        t1[:], d[:], -1.0, 0.0, ALU.add, ALU.min)
                    eh = misc.tile([128, 1], F32, tag=f"eh{plo}",
                                   name=f"eh{plo}")
                    je = jkp.tile([128, pw], F32, tag=f"je{plo}",
                                  name=f"je{plo}")
                    nc.vector.tensor_tensor(je[:], t1[:], md_sb[:, sl],
                                            ALU.mult)
                    nc.vector.reduce_sum(eh[:], je[:],
                                         axis=mybir.AxisListType.X)
                    acc_e.append(eh)

            out_sb = misc.tile([128, 2], F32, tag="osb", name="osb")
            hsum = misc.tile([128, 2], F32, tag="hsum", name="hsum")
            esum = misc.tile([128, 2], F32, tag="esum", name="esum")
            nc.vector.tensor_tensor(hsum[:, 0:1], acc_h[0][:], acc_h[1][:],
                                    ALU.add)
            nc.vector.tensor_tensor(esum[:, 0:1], acc_e[0][:], acc_e[1][:],
                                    ALU.add)
            for i in range(2, len(acc_h) - 1):
                nc.vector.tensor_tensor(hsum[:, 0:1], hsum[:, 0:1],
                                        acc_h[i][:], ALU.add)
                nc.vector.tensor_tensor(esum[:, 0:1], esum[:, 0:1],
                                        acc_e[i][:], ALU.add)
            nc.vector.tensor_tensor(out_sb[:, 0:1], hsum[:, 0:1],
                                    acc_h[-1][:], ALU.add)
            nc.vector.tensor_tensor(out_sb[:, 1:2], esum[:, 0:1],
                                    acc_e[-1][:], ALU.add)
            nc.sync.dma_start(out[:], out_sb[:])

    nc.compile()
    return nc


def _host_inputs(x: np.ndarray):
    """Shard + transpose + cast x into per-core input maps."""
    f = np.ascontiguousarray(x.reshape(N, K))
    groups = np.arange(N) // BK
    cols = np.arange(N)

    in_maps = []
    for c in range(N_CORES):
        ftc = np.ascontiguousarray(
            f[:, c * KC:(c + 1) * KC].T).astype(ml_dtypes.bfloat16)
        rows = c * 128 + np.arange(128)
        g_r = groups[rows]
        same = ((g_r[:, None] == groups[None, :]) &
                (rows[:, None] != cols[None, :])).astype(np.float32)
        diff = (g_r[:, None] != groups[None, :]).astype(np.float32)
        # upper-triangle pair weighting: drop below-block pairs, count
        # above-block pairs twice (dist is symmetric)
        w = np.where(cols < 128 * c, 0.0,
                     np.where(cols < 128 * (c + 1), 1.0, 2.0))
        diff = diff * w[None, :].astype(np.float32)
        em = np.zeros((128, 8), dtype=np.float32)
        em[:, c] = 1.0
        in_maps.append({
            "ft": ftc.reshape(KT, 128, N),
            "mask_same": same,
            "mask_diff": diff,
            "emask": em,
        })
    return in_maps


def kernel(x: np.ndarray):
    if "nc" not in _CACHE:
        _CACHE["nc"] = _build_nc()
    nc = _CACHE["nc"]

    in_maps = _host_inputs(x)
    res = bass_utils.run_bass_kernel_spmd(
        nc, in_maps, core_ids=list(range(N_CORES)))

    total_h = 0.0
    total_e = 0.0
    for c in range(N_CORES):
        o = res.results[c]["out"].astype(np.float64)
        total_h += o[:, 0].sum()
        total_e += o[:, 1].sum()

    # reference: 2 * (0.5 * sum_same dist) / (N * (BK - 1))
    #            2 * sum_diff relu(1 - dist) / (N * (N // BK - 1))
    # device accumulates sum(min(dist-1, 0) * mask_diff) = -heter partial
    homo = total_h / (N * (BK - 1))
    heter = -2.0 * total_e / (N * (N // BK - 1))
    return (np.float32(homo), np.float32(heter))


# revision 24
# speedup vs baseline: 1.1652x; 1.1652x over previous
"""Trainium2 Bass kernel for nn_MetricLoss (pairwise-distance metric loss).

Computation (reference):
    f = x.reshape(1024, 49152)
    G = f @ f.T                      (103 GFLOP Gram matrix)
    dist = sq_i + sq_j - 2 G         (the relu(dist) only binds on the
                                      diagonal, which both masks zero out)
    loss_homo  = 0.5 * sum(same-group dist)
    loss_heter = sum(cross-group relu(1 - dist))

Distribution (8 NeuronCores, one TRN2 chip):
    K-parallel with symmetry: core c holds f[:, c*6144:(c+1)*6144].T as a
    [48, 128, 1024] bf16 tensor (k-major tiles, fully SBUF-resident). Since
    the Gram matrix and both losses are symmetric, each core computes only
    the upper-triangle 128x128 blocks of its partial Gram (36/64 of the
    matmul work) via PSUM-accumulated chains over 4 column chunks, and the
    cross-block loss terms are counted once with weight 2 via a per-core
    weighted mask — the lower triangle is never materialized. Chunked bf16
    ReduceScatters give core c the upper-triangle part of full-K Gram rows
    [128c:128c+128]. Row norms sq are computed in fp32 on the otherwise-idle
    Scalar + Vector + GpSimd engines and summed across cores with a tiny
    fp32 AllReduce issued first (it also absorbs the collective-engine cold
    start). A fused DVE epilogue computes the masked hinge sums per chunk as
    each ReduceScatter lands; the host sums 8x[128,2] partials and
    normalizes.
"""

import numpy as np
import ml_dtypes

import concourse.bass as bass
import concourse.bacc as bacc
import concourse.tile as tile
import concourse.mybir as mybir
import concourse.bass_isa as bass_isa
from concourse.tile_rust import add_dep_helper
from concourse import bass_utils

F32 = mybir.dt.float32
BF16 = mybir.dt.bfloat16
ALU = mybir.AluOpType
AF = mybir.ActivationFunctionType

N_CORES = 8
N = 1024            # batch (rows of f)
K = 64 * 768        # 49152 features per sample
KC = K // N_CORES   # 6144 features per core
KT = KC // 128      # 48 k-tiles of 128 per core
BK = 8              # samples per class group
MB = N // 128       # 8 row blocks

CWS = [384, 256, 256, 128]          # column-chunk widths
OFF = [0, 384, 640, 896]            # chunk column offsets
NJ = len(CWS)
# process biggest chunks first (hides the input DMA), smallest last
CH_ORDER = [2, 1, 0, 3]

_CACHE = {}


def _chunk_chains(j):
    """Upper-triangle chains for chunk j: (m, col_off, width)."""
    chains = []
    for m in range(MB):
        lo = max(OFF[j], 128 * m)
        hi = OFF[j] + CWS[j]
        if lo < hi:
            chains.append((m, lo, hi - lo))
    return chains


def _build_nc():
    nc = bacc.Bacc("TRN2", target_bir_lowering=False, debug=False,
                   num_devices=N_CORES)

    ft = nc.dram_tensor("ft", [KT, 128, N], BF16, kind="ExternalInput").ap()
    mask_same = nc.dram_tensor("mask_same", [128, N], F32,
                               kind="ExternalInput").ap()
    # weighted cross-group mask: 0 below own block, 1 within own block,
    # 2 above it (upper-triangle pair counting)
    mask_diff = nc.dram_tensor("mask_diff", [128, N], F32,
                               kind="ExternalInput").ap()
    emask = nc.dram_tensor("emask", [128, 8], F32, kind="ExternalInput").ap()
    out = nc.dram_tensor("out", [128, 2], F32, kind="ExternalOutput").ap()

    rg = [list(range(N_CORES))]

    with tile.TileContext(nc) as tc:
        with (
            tc.tile_pool(name="ftp", bufs=1) as ftp,
            tc.tile_pool(name="misc", bufs=1) as misc,
            tc.tile_pool(name="gcopy", bufs=4) as gcp,
            tc.tile_pool(name="sqt", bufs=3) as sqtp,
            tc.tile_pool(name="junk", bufs=1) as jkp,
            tc.tile_pool(name="psum", bufs=8, space="PSUM") as psp,
            tc.tile_pool(name="dram", bufs=1, space="DRAM") as drp,
        ):
            # ---- load inputs to SBUF ----
            ft_sb = []
            for k in range(KT):
                t = ftp.tile([128, N], BF16, tag=f"ft{k}", name=f"ft{k}")
                nc.sync.dma_start(t[:], ft[k])
                ft_sb.append(t)

            ms_sb = misc.tile([128, N], F32, tag="ms", name="ms")
            md_sb = misc.tile([128, N], F32, tag="md", name="md")
            em_sb = misc.tile([128, 8], F32, tag="em", name="em")
            nc.sync.dma_start(ms_sb[:], mask_same[:])
            nc.sync.dma_start(md_sb[:], mask_diff[:])
            nc.sync.dma_start(em_sb[:], emask[:])

            # ---- sq pipeline on ACT (square) + DVE (accumulate) ----
            acc = misc.tile([128, N], F32, tag="acc", name="acc")
            nc.vector.memset(acc[:], 0.0)
            for k in range(KT):
                sqt = sqtp.tile([128, N], F32, tag="sqt", name=f"sqt{k}")
                nc.scalar.activation(sqt[:], ft_sb[k][:], AF.Square)
                nc.vector.tensor_tensor(acc[:], acc[:], sqt[:], ALU.add)

            sqb = drp.tile([1, N], F32, tag="sqb", name="sqb")
            sq_ar = drp.tile([1, N], F32, tag="sqar", name="sq_ar")
            # cross-partition reduce of acc without touching the gpsimd
            # queue: error-free bf16 hi/lo split, 16 bf16 DMA transposes,
            # fp32 DVE reduces
            acc_hi = misc.tile([128, N], BF16, tag="ahi", name="acc_hi")
            acc_lo = misc.tile([128, N], BF16, tag="alo", name="acc_lo")
            lo_f = misc.tile([128, N], F32, tag="alof", name="lo_f")
            nc.vector.tensor_copy(acc_hi[:], acc[:])
            nc.vector.tensor_tensor(lo_f[:], acc[:], acc_hi[:], ALU.subtract)
            nc.vector.tensor_copy(acc_lo[:], lo_f[:])
            hiT = misc.tile([128, N], BF16, tag="hiT", name="hiT")
            loT = misc.tile([128, N], BF16, tag="loT", name="loT")
            sq_hi = misc.tile([128, 8], F32, tag="sqh", name="sq_hi")
            sq_lo = misc.tile([128, 8], F32, tag="sql", name="sq_lo")
            sqc = misc.tile([128, 8], F32, tag="sqc", name="sqc")
            for cix in range(MB):
                tsl = slice(cix * 128, (cix + 1) * 128)
                nc.sync.dma_start(hiT[:, tsl], acc_hi[:, tsl], transpose=True)
                nc.sync.dma_start(loT[:, tsl], acc_lo[:, tsl], transpose=True)
                nc.vector.reduce_sum(sq_hi[:, cix:cix + 1], hiT[:, tsl],
                                     axis=mybir.AxisListType.X)
                nc.vector.reduce_sum(sq_lo[:, cix:cix + 1], loT[:, tsl],
                                     axis=mybir.AxisListType.X)
                nc.vector.tensor_tensor(sqc[:, cix:cix + 1],
                                        sq_hi[:, cix:cix + 1],
                                        sq_lo[:, cix:cix + 1], ALU.add)
                nc.sync.dma_start(sqb[0:1, cix * 128:(cix + 1) * 128],
                                  sqc[:, cix:cix + 1])
            # warmup collective: rings the CC doorbell at t~5us so the
            # collective engine's cold start overlaps the compute
            warm_sb = misc.tile([1, 8], F32, tag="wsb", name="warm_sb")
            nc.vector.memset(warm_sb[:], 0.0)
            warm_in = drp.tile([1, 8], F32, tag="wi", name="warm_in")
            warm_out = drp.tile([1, 8], F32, tag="wo", name="warm_out")
            nc.sync.dma_start(warm_in[:], warm_sb[:])
            warm_cc = nc.gpsimd.collective_compute(
                "AllReduce", ALU.add, replica_groups=rg,
                ins=[warm_in.opt()], outs=[warm_out.opt()])

            bounce = {}
            rs = {}
            for j in range(NJ):
                bounce[j] = drp.tile([N, CWS[j]], BF16, tag=f"bnc{j}",
                                     name=f"bnc{j}")
                rs[j] = drp.tile([128, CWS[j]], BF16, tag=f"rs{j}",
                                 name=f"rs{j}")

            # zero the never-written (lower-triangle) bounce regions so the
            # ReduceScatter output is finite everywhere
            zero_sb = misc.tile([128, 384], BF16, tag="z", name="zero_sb")
            nc.vector.memset(zero_sb[:], 0.0)
            for j in range(NJ):
                nb = (OFF[j] + CWS[j]) // 128
                for m in range(nb, MB):
                    nc.sync.dma_start(
                        bounce[j][m * 128:(m + 1) * 128, :],
                        zero_sb[:, 0:CWS[j]])
                for (m, lo, w) in _chunk_chains(j):
                    if lo > OFF[j]:
                        nc.sync.dma_start(
                            bounce[j][m * 128:(m + 1) * 128, 0:lo - OFF[j]],
                            zero_sb[:, 0:lo - OFF[j]])

            # ---- upper-triangle partial Gram ----
            # collectives emitted + pinned in order:
            # warm, RS[c2], RS[c1], AR_sq, RS[c0], RS[c3]
            prev_cc = warm_cc
            for idx, j in enumerate(CH_ORDER):
                chs = _chunk_chains(j)
                ptiles = {}
                for (m, lo, w) in chs:
                    ptiles[m] = psp.tile([128, w], F32, tag="chain",
                                         name=f"ch{j}_{m}")
                for k in range(KT):
                    for (m, lo, w) in chs:
                        nc.tensor.matmul(
                            ptiles[m][:],
                            lhsT=ft_sb[k][:, m * 128:(m + 1) * 128],
                            rhs=ft_sb[k][:, lo:lo + w],
                            start=(k == 0),
                            stop=(k == KT - 1),
                        )
                for (m, lo, w) in chs:
                    g = gcp.tile([128, w], BF16, tag="g", name=f"g{j}_{m}")
                    nc.vector.tensor_copy(g[:], ptiles[m][:])
                    nc.sync.dma_start(
                        bounce[j][m * 128:(m + 1) * 128,
                                  lo - OFF[j]:lo - OFF[j] + w], g[:])
                if idx == 2:
                    # sq AllReduce slotted mid-chain (its input is ready
                    # by then; results feed the per-chunk epilogues)
                    ar_cc = nc.gpsimd.collective_compute(
                        "AllReduce", ALU.add, replica_groups=rg,
                        ins=[sqb.opt()], outs=[sq_ar.opt()])
                    add_dep_helper(ar_cc.ins, prev_cc.ins, False, "cc order")
                    prev_cc = ar_cc
                rs_cc = nc.gpsimd.collective_compute(
                    "ReduceScatter", ALU.add, replica_groups=rg,
                    ins=[bounce[j].opt()], outs=[rs[j].opt()])
                add_dep_helper(rs_cc.ins, prev_cc.ins, False, "cc order")
                prev_cc = rs_cc

            # ---- sq_row + sq_col broadcast ----
            flat_sb = misc.tile([1, N], F32, tag="flat", name="flat")
            nc.sync.dma_start(flat_sb[:], sq_ar[:])
            S_all = misc.tile([128, 8], F32, tag="S", name="S")
            for b in range(MB):
                nc.sync.dma_start(S_all[:, b:b + 1],
                                  sq_ar[0:1, b * 128:(b + 1) * 128])
            sq_row = misc.tile([128, 1], F32, tag="sqr", name="sqr")
            junk8 = misc.tile([128, 8], F32, tag="jk8", name="junk8")
            nc.vector.tensor_tensor(junk8[:], S_all[:], em_sb[:], ALU.mult)
            nc.vector.reduce_sum(sq_row[:], junk8[:],
                                 axis=mybir.AxisListType.X)
            ones = misc.tile([1, 128], F32, tag="ones", name="ones")
            nc.vector.memset(ones[:], 1.0)
            # PE is idle after the passes; PSUM chain slots are free
            Bs = []
            for h in range(2):
                Bh = psp.tile([128, 512], F32, tag="chain", name=f"B{h}")
                nc.tensor.matmul(Bh[:], lhsT=ones[:],
                                 rhs=flat_sb[0:1, h * 512:(h + 1) * 512],
                                 start=True, stop=True)
                Bs.append(Bh)

            # ---- per-chunk epilogue (fires as each RS lands) ----
            G_sb = misc.tile([128, N], BF16, tag="G", name="G")
            acc_h = []
            acc_e = []
            for j in CH_ORDER:
                W = CWS[j]
                nc.sync.dma_start(G_sb[:, OFF[j]:OFF[j] + W], rs[j][:, :])
                # split at the 512 boundary so each piece maps into one
                # B half
                pieces = []
                lo = OFF[j]
                hi = OFF[j] + W
                while lo < hi:
                    p_hi = min(hi, (lo // 512 + 1) * 512)
                    pieces.append((lo, p_hi - lo))
                    lo = p_hi
                for (plo, pw) in pieces:
                    sl = slice(plo, plo + pw)
                    t0 = jkp.tile([128, pw], F32, tag=f"t0_{plo}",
                                  name=f"t0_{plo}")
                    nc.vector.tensor_scalar_mul(t0[:], G_sb[:, sl], -2.0)
                    d = jkp.tile([128, pw], F32, tag=f"d{plo}",
                                 name=f"d{plo}")
                    Bsl = Bs[plo // 512][:, plo % 512:plo % 512 + pw]
                    nc.vector.scalar_tensor_tensor(
                        out=d[:], in0=t0[:], scalar=sq_row[:], in1=Bsl,
                        op0=ALU.add, op1=ALU.add)
                    ah = misc.tile([128, 1], F32, tag=f"ah{plo}",
                                   name=f"ah{plo}")
                    jh = jkp.tile([128, pw], F32, tag=f"jh{plo}",
                                  name=f"jh{plo}")
                    nc.vector.tensor_tensor(jh[:], d[:], ms_sb[:, sl],
                                            ALU.mult)
                    nc.vector.reduce_sum(ah[:], jh[:],
                                         axis=mybir.AxisListType.X)
                    acc_h.append(ah)
                    # min(d-1, 0) = -relu(1-d); heter partial =
                    # -sum(mask * that), negation applied on the host
                    t1 = jkp.tile([128, pw], F32, tag=f"t1_{plo}",
                                  name=f"t1_{plo}")
                    nc.vector.tensor_scalar(
                        t1[:], d[:], -1.0, 0.0, ALU.add, ALU.min)
                    eh = misc.tile([128, 1], F32, tag=f"eh{plo}",
                                   name=f"eh{plo}")
                    je = jkp.tile([128, pw], F32, tag=f"je{plo}",
                                  name=f"je{plo}")
                    nc.vector.tensor_tensor(je[:], t1[:], md_sb[:, sl],
                                            ALU.mult)
                    nc.vector.reduce_sum(eh[:], je[:],
                                         axis=mybir.AxisListType.X)
                    acc_e.append(eh)

            out_sb = misc.tile([128, 2], F32, tag="osb", name="osb")
            hsum = misc.tile([128, 2], F32, tag="hsum", name="hsum")
            esum = misc.tile([128, 2], F32, tag="esum", name="esum")
            nc.vector.tensor_tensor(hsum[:, 0:1], acc_h[0][:], acc_h[1][:],
                                    ALU.add)
            nc.vector.tensor_tensor(esum[:, 0:1], acc_e[0][:], acc_e[1][:],
                                    ALU.add)
            for i in range(2, len(acc_h) - 1):
                nc.vector.tensor_tensor(hsum[:, 0:1], hsum[:, 0:1],
                                        acc_h[i][:], ALU.add)
                nc.vector.tensor_tensor(esum[:, 0:1], esum[:, 0:1],
                                        acc_e[i][:], ALU.add)
            nc.vector.tensor_tensor(out_sb[:, 0:1], hsum[:, 0:1],
                                    acc_h[-1][:], ALU.add)
            nc.vector.tensor_tensor(out_sb[:, 1:2], esum[:, 0:1],
                                    acc_e[-1][:], ALU.add)
            nc.sync.dma_start(out[:], out_sb[:])

    nc.compile()
    return nc


def _host_inputs(x: np.ndarray):
    """Shard + transpose + cast x into per-core input maps."""
    f = np.ascontiguousarray(x.reshape(N, K))
    groups = np.arange(N) // BK
    cols = np.arange(N)

    in_maps = []
    for c in range(N_CORES):
        ftc = np.ascontiguousarray(
            f[:, c * KC:(c + 1) * KC].T).astype(ml_dtypes.bfloat16)
        rows = c * 128 + np.arange(128)
        g_r = groups[rows]
        same = ((g_r[:, None] == groups[None, :]) &
                (rows[:, None] != cols[None, :])).astype(np.float32)
        diff = (g_r[:, None] != groups[None, :]).astype(np.float32)
        # upper-triangle pair weighting: drop below-block pairs, count
        # above-block pairs twice (dist is symmetric)
        w = np.where(cols < 128 * c, 0.0,
                     np.where(cols < 128 * (c + 1), 1.0, 2.0))
        diff = diff * w[None, :].astype(np.float32)
        em = np.zeros((128, 8), dtype=np.float32)
        em[:, c] = 1.0
        in_maps.append({
            "ft": ftc.reshape(KT, 128, N),
            "mask_same": same,
            "mask_diff": diff,
            "emask": em,
        })
    return in_maps


def kernel(x: np.ndarray):
    if "nc" not in _CACHE:
        _CACHE["nc"] = _build_nc()
    nc = _CACHE["nc"]

    in_maps = _host_inputs(x)
    res = bass_utils.run_bass_kernel_spmd(
        nc, in_maps, core_ids=list(range(N_CORES)))

    total_h = 0.0
    total_e = 0.0
    for c in range(N_CORES):
        o = res.results[c]["out"].astype(np.float64)
        total_h += o[:, 0].sum()
        total_e += o[:, 1].sum()

    # reference: 2 * (0.5 * sum_same dist) / (N * (BK - 1))
    #            2 * sum_diff relu(1 - dist) / (N * (N // BK - 1))
    # device accumulates sum(min(dist-1, 0) * mask_diff) = -heter partial
    homo = total_h / (N * (BK - 1))
    heter = -2.0 * total_e / (N * (N // BK - 1))
    return (np.float32(homo), np.float32(heter))


# revision 25
# speedup vs baseline: 1.1691x; 1.0034x over previous
"""Trainium2 Bass kernel for nn_MetricLoss (pairwise-distance metric loss).

Computation (reference):
    f = x.reshape(1024, 49152)
    G = f @ f.T                      (103 GFLOP Gram matrix)
    dist = sq_i + sq_j - 2 G         (the relu(dist) only binds on the
                                      diagonal, which both masks zero out)
    loss_homo  = 0.5 * sum(same-group dist)
    loss_heter = sum(cross-group relu(1 - dist))

Distribution (8 NeuronCores, one TRN2 chip):
    K-parallel with symmetry: core c holds f[:, c*6144:(c+1)*6144].T as a
    [48, 128, 1024] bf16 tensor (k-major tiles, fully SBUF-resident). Since
    the Gram matrix and both losses are symmetric, each core computes only
    the upper-triangle 128x128 blocks of its partial Gram (36/64 of the
    matmul work) via PSUM-accumulated chains over 4 column chunks, and the
    cross-block loss terms are counted once with weight 2 via a per-core
    weighted mask — the lower triangle is never materialized. Chunked bf16
    ReduceScatters give core c the upper-triangle part of full-K Gram rows
    [128c:128c+128]. Row norms sq are computed in fp32 on the otherwise-idle
    Scalar + Vector + GpSimd engines and summed across cores with a tiny
    fp32 AllReduce issued first (it also absorbs the collective-engine cold
    start). A fused DVE epilogue computes the masked hinge sums per chunk as
    each ReduceScatter lands; the host sums 8x[128,2] partials and
    normalizes.
"""

import numpy as np
import ml_dtypes

import concourse.bass as bass
import concourse.bacc as bacc
import concourse.tile as tile
import concourse.mybir as mybir
import concourse.bass_isa as bass_isa
from concourse.tile_rust import add_dep_helper
from concourse import bass_utils

F32 = mybir.dt.float32
BF16 = mybir.dt.bfloat16
ALU = mybir.AluOpType
AF = mybir.ActivationFunctionType

N_CORES = 8
N = 1024            # batch (rows of f)
K = 64 * 768        # 49152 features per sample
KC = K // N_CORES   # 6144 features per core
KT = KC // 128      # 48 k-tiles of 128 per core
BK = 8              # samples per class group
MB = N // 128       # 8 row blocks

CWS = [384, 256, 256, 128]          # column-chunk widths
OFF = [0, 384, 640, 896]            # chunk column offsets
NJ = len(CWS)
# process biggest chunks first (hides the input DMA), smallest last
CH_ORDER = [2, 1, 0, 3]

_CACHE = {}


def _chunk_chains(j):
    """Upper-triangle chains for chunk j: (m, col_off, width)."""
    chains = []
    for m in range(MB):
        lo = max(OFF[j], 128 * m)
        hi = OFF[j] + CWS[j]
        if lo < hi:
            chains.append((m, lo, hi - lo))
    return chains


def _build_nc():
    nc = bacc.Bacc("TRN2", target_bir_lowering=False, debug=False,
                   num_devices=N_CORES)

    ft = nc.dram_tensor("ft", [KT, 128, N], BF16, kind="ExternalInput").ap()
    mask_same = nc.dram_tensor("mask_same", [128, N], F32,
                               kind="ExternalInput").ap()
    # weighted cross-group mask: 0 below own block, 1 within own block,
    # 2 above it (upper-triangle pair counting)
    mask_diff = nc.dram_tensor("mask_diff", [128, N], F32,
                               kind="ExternalInput").ap()
    emask = nc.dram_tensor("emask", [128, 8], F32, kind="ExternalInput").ap()
    out = nc.dram_tensor("out", [128, 2], F32, kind="ExternalOutput").ap()

    rg = [list(range(N_CORES))]

    with tile.TileContext(nc) as tc:
        with (
            tc.tile_pool(name="ftp", bufs=1) as ftp,
            tc.tile_pool(name="misc", bufs=1) as misc,
            tc.tile_pool(name="gcopy", bufs=4) as gcp,
            tc.tile_pool(name="sqt", bufs=3) as sqtp,
            tc.tile_pool(name="junk", bufs=1) as jkp,
            tc.tile_pool(name="psum", bufs=8, space="PSUM") as psp,
            tc.tile_pool(name="dram", bufs=1, space="DRAM") as drp,
        ):
            # ---- load inputs to SBUF ----
            ft_sb = []
            for k in range(KT):
                t = ftp.tile([128, N], BF16, tag=f"ft{k}", name=f"ft{k}")
                nc.sync.dma_start(t[:], ft[k])
                ft_sb.append(t)

            ms_sb = misc.tile([128, N], F32, tag="ms", name="ms")
            md_sb = misc.tile([128, N], F32, tag="md", name="md")
            em_sb = misc.tile([128, 8], F32, tag="em", name="em")
            nc.sync.dma_start(ms_sb[:], mask_same[:])
            nc.sync.dma_start(md_sb[:], mask_diff[:])
            nc.sync.dma_start(em_sb[:], emask[:])

            # ---- sq pipeline on ACT (square) + DVE (accumulate) ----
            acc = misc.tile([128, N], F32, tag="acc", name="acc")
            nc.vector.memset(acc[:], 0.0)
            for k in range(KT):
                sqt = sqtp.tile([128, N], F32, tag="sqt", name=f"sqt{k}")
                nc.scalar.activation(sqt[:], ft_sb[k][:], AF.Square)
                nc.vector.tensor_tensor(acc[:], acc[:], sqt[:], ALU.add)

            sqb = drp.tile([1, N], F32, tag="sqb", name="sqb")
            sq_ar = drp.tile([1, N], F32, tag="sqar", name="sq_ar")
            # cross-partition reduce of acc without touching the gpsimd
            # queue: error-free bf16 hi/lo split, 16 bf16 DMA transposes,
            # fp32 DVE reduces
            acc_hi = misc.tile([128, N], BF16, tag="ahi", name="acc_hi")
            acc_lo = misc.tile([128, N], BF16, tag="alo", name="acc_lo")
            lo_f = misc.tile([128, N], F32, tag="alof", name="lo_f")
            nc.vector.tensor_copy(acc_hi[:], acc[:])
            nc.vector.tensor_tensor(lo_f[:], acc[:], acc_hi[:], ALU.subtract)
            nc.vector.tensor_copy(acc_lo[:], lo_f[:])
            hiT = misc.tile([128, N], BF16, tag="hiT", name="hiT")
            loT = misc.tile([128, N], BF16, tag="loT", name="loT")
            sq_hi = misc.tile([128, 8], F32, tag="sqh", name="sq_hi")
            sq_lo = misc.tile([128, 8], F32, tag="sql", name="sq_lo")
            sqc = misc.tile([128, 8], F32, tag="sqc", name="sqc")
            for cix in range(MB):
                tsl = slice(cix * 128, (cix + 1) * 128)
                nc.sync.dma_start(hiT[:, tsl], acc_hi[:, tsl], transpose=True)
                nc.sync.dma_start(loT[:, tsl], acc_lo[:, tsl], transpose=True)
                nc.vector.reduce_sum(sq_hi[:, cix:cix + 1], hiT[:, tsl],
                                     axis=mybir.AxisListType.X)
                nc.vector.reduce_sum(sq_lo[:, cix:cix + 1], loT[:, tsl],
                                     axis=mybir.AxisListType.X)
                nc.vector.tensor_tensor(sqc[:, cix:cix + 1],
                                        sq_hi[:, cix:cix + 1],
                                        sq_lo[:, cix:cix + 1], ALU.add)
                nc.sync.dma_start(sqb[0:1, cix * 128:(cix + 1) * 128],
                                  sqc[:, cix:cix + 1])

            bounce = {}
            rs = {}
            for j in range(NJ):
                bounce[j] = drp.tile([N, CWS[j]], BF16, tag=f"bnc{j}",
                                     name=f"bnc{j}")
                rs[j] = drp.tile([128, CWS[j]], BF16, tag=f"rs{j}",
                                 name=f"rs{j}")

            # zero the never-written (lower-triangle) bounce regions so the
            # ReduceScatter output is finite everywhere
            zero_sb = misc.tile([128, 384], BF16, tag="z", name="zero_sb")
            nc.vector.memset(zero_sb[:], 0.0)
            for j in range(NJ):
                nb = (OFF[j] + CWS[j]) // 128
                for m in range(nb, MB):
                    nc.sync.dma_start(
                        bounce[j][m * 128:(m + 1) * 128, :],
                        zero_sb[:, 0:CWS[j]])
                for (m, lo, w) in _chunk_chains(j):
                    if lo > OFF[j]:
                        nc.sync.dma_start(
                            bounce[j][m * 128:(m + 1) * 128, 0:lo - OFF[j]],
                            zero_sb[:, 0:lo - OFF[j]])

            # ---- upper-triangle partial Gram ----
            # collectives emitted + pinned in order:
            # RS[c2], RS[c1], AR_sq, RS[c0], RS[c3]
            prev_cc = None
            for idx, j in enumerate(CH_ORDER):
                chs = _chunk_chains(j)
                ptiles = {}
                for (m, lo, w) in chs:
                    ptiles[m] = psp.tile([128, w], F32, tag="chain",
                                         name=f"ch{j}_{m}")
                for k in range(KT):
                    for (m, lo, w) in chs:
                        nc.tensor.matmul(
                            ptiles[m][:],
                            lhsT=ft_sb[k][:, m * 128:(m + 1) * 128],
                            rhs=ft_sb[k][:, lo:lo + w],
                            start=(k == 0),
                            stop=(k == KT - 1),
                        )
                for (m, lo, w) in chs:
                    g = gcp.tile([128, w], BF16, tag="g", name=f"g{j}_{m}")
                    nc.vector.tensor_copy(g[:], ptiles[m][:])
                    nc.sync.dma_start(
                        bounce[j][m * 128:(m + 1) * 128,
                                  lo - OFF[j]:lo - OFF[j] + w], g[:])
                if idx == 2:
                    # sq AllReduce slotted mid-chain (its input is ready
                    # by then; results feed the per-chunk epilogues)
                    ar_cc = nc.gpsimd.collective_compute(
                        "AllReduce", ALU.add, replica_groups=rg,
                        ins=[sqb.opt()], outs=[sq_ar.opt()])
                    if prev_cc is not None:
                        add_dep_helper(ar_cc.ins, prev_cc.ins, False,
                                       "cc order")
                    prev_cc = ar_cc
                rs_cc = nc.gpsimd.collective_compute(
                    "ReduceScatter", ALU.add, replica_groups=rg,
                    ins=[bounce[j].opt()], outs=[rs[j].opt()])
                if prev_cc is not None:
                    add_dep_helper(rs_cc.ins, prev_cc.ins, False, "cc order")
                prev_cc = rs_cc

            # ---- sq_row + sq_col broadcast ----
            flat_sb = misc.tile([1, N], F32, tag="flat", name="flat")
            nc.sync.dma_start(flat_sb[:], sq_ar[:])
            S_all = misc.tile([128, 8], F32, tag="S", name="S")
            for b in range(MB):
                nc.sync.dma_start(S_all[:, b:b + 1],
                                  sq_ar[0:1, b * 128:(b + 1) * 128])
            sq_row = misc.tile([128, 1], F32, tag="sqr", name="sqr")
            junk8 = misc.tile([128, 8], F32, tag="jk8", name="junk8")
            nc.vector.tensor_tensor(junk8[:], S_all[:], em_sb[:], ALU.mult)
            nc.vector.reduce_sum(sq_row[:], junk8[:],
                                 axis=mybir.AxisListType.X)
            ones = misc.tile([1, 128], F32, tag="ones", name="ones")
            nc.vector.memset(ones[:], 1.0)
            # PE is idle after the passes; PSUM chain slots are free
            Bs = []
            for h in range(2):
                Bh = psp.tile([128, 512], F32, tag="chain", name=f"B{h}")
                nc.tensor.matmul(Bh[:], lhsT=ones[:],
                                 rhs=flat_sb[0:1, h * 512:(h + 1) * 512],
                                 start=True, stop=True)
                Bs.append(Bh)

            # ---- per-chunk epilogue (fires as each RS lands) ----
            G_sb = misc.tile([128, N], BF16, tag="G", name="G")
            acc_h = []
            acc_e = []
            for j in CH_ORDER:
                W = CWS[j]
                nc.sync.dma_start(G_sb[:, OFF[j]:OFF[j] + W], rs[j][:, :])
                # split at the 512 boundary so each piece maps into one
                # B half
                pieces = []
                lo = OFF[j]
                hi = OFF[j] + W
                while lo < hi:
                    p_hi = min(hi, (lo // 512 + 1) * 512)
                    pieces.append((lo, p_hi - lo))
                    lo = p_hi
                for (plo, pw) in pieces:
                    sl = slice(plo, plo + pw)
                    t0 = jkp.tile([128, pw], F32, tag=f"t0_{plo}",
                                  name=f"t0_{plo}")
                    nc.vector.tensor_scalar_mul(t0[:], G_sb[:, sl], -2.0)
                    d = jkp.tile([128, pw], F32, tag=f"d{plo}",
                                 name=f"d{plo}")
                    Bsl = Bs[plo // 512][:, plo % 512:plo % 512 + pw]
                    nc.vector.scalar_tensor_tensor(
                        out=d[:], in0=t0[:], scalar=sq_row[:], in1=Bsl,
                        op0=ALU.add, op1=ALU.add)
                    ah = misc.tile([128, 1], F32, tag=f"ah{plo}",
                                   name=f"ah{plo}")
                    jh = jkp.tile([128, pw], F32, tag=f"jh{plo}",
                                  name=f"jh{plo}")
                    nc.vector.tensor_tensor(jh[:], d[:], ms_sb[:, sl],
                                            ALU.mult)
                    nc.vector.reduce_sum(ah[:], jh[:],
                                         axis=mybir.AxisListType.X)
                    acc_h.append(ah)
                    # min(d-1, 0) = -relu(1-d); heter partial =
                    # -sum(mask * that), negation applied on the host
                    t1 = jkp.tile([128, pw], F32, tag=f"t1_{plo}",
                                  name=f"t1_{plo}")
                    nc.vector.tensor_scalar(
                        t1[:], d[:], -1.0, 0.0, ALU.add, ALU.min)
                    eh = misc.tile([128, 1], F32, tag=f"eh{plo}",
                                   name=f"eh{plo}")
                    je = jkp.tile([128, pw], F32, tag=f"je{plo}",
                                  name=f"je{plo}")
                    nc.vector.tensor_tensor(je[:], t1[:], md_sb[:, sl],
                                            ALU.mult)
                    nc.vector.reduce_sum(eh[:], je[:],
                                         axis=mybir.AxisListType.X)
                    acc_e.append(eh)

            out_sb = misc.tile([128, 2], F32, tag="osb", name="osb")
            hsum = misc.tile([128, 2], F32, tag="hsum", name="hsum")
            esum = misc.tile([128, 2], F32, tag="esum", name="esum")
            nc.vector.tensor_tensor(hsum[:, 0:1], acc_h[0][:], acc_h[1][:],
                                    ALU.add)
            nc.vector.tensor_tensor(esum[:, 0:1], acc_e[0][:], acc_e[1][:],
                                    ALU.add)
            for i in range(2, len(acc_h) - 1):
                nc.vector.tensor_tensor(hsum[:, 0:1], hsum[:, 0:1],
                                        acc_h[i][:], ALU.add)
                nc.vector.tensor_tensor(esum[:, 0:1], esum[:, 0:1],
                                        acc_e[i][:], ALU.add)
            nc.vector.tensor_tensor(out_sb[:, 0:1], hsum[:, 0:1],
                                    acc_h[-1][:], ALU.add)
            nc.vector.tensor_tensor(out_sb[:, 1:2], esum[:, 0:1],
                                    acc_e[-1][:], ALU.add)
            nc.sync.dma_start(out[:], out_sb[:])

    nc.compile()
    return nc


def _host_inputs(x: np.ndarray):
    """Shard + transpose + cast x into per-core input maps."""
    f = np.ascontiguousarray(x.reshape(N, K))
    groups = np.arange(N) // BK
    cols = np.arange(N)

    in_maps = []
    for c in range(N_CORES):
        ftc = np.ascontiguousarray(
            f[:, c * KC:(c + 1) * KC].T).astype(ml_dtypes.bfloat16)
        rows = c * 128 + np.arange(128)
        g_r = groups[rows]
        same = ((g_r[:, None] == groups[None, :]) &
                (rows[:, None] != cols[None, :])).astype(np.float32)
        diff = (g_r[:, None] != groups[None, :]).astype(np.float32)
        # upper-triangle pair weighting: drop below-block pairs, count
        # above-block pairs twice (dist is symmetric)
        w = np.where(cols < 128 * c, 0.0,
                     np.where(cols < 128 * (c + 1), 1.0, 2.0))
        diff = diff * w[None, :].astype(np.float32)
        em = np.zeros((128, 8), dtype=np.float32)
        em[:, c] = 1.0
        in_maps.append({
            "ft": ftc.reshape(KT, 128, N),
            "mask_same": same,
            "mask_diff": diff,
            "emask": em,
        })
    return in_maps


def kernel(x: np.ndarray):
    if "nc" not in _CACHE:
        _CACHE["nc"] = _build_nc()
    nc = _CACHE["nc"]

    in_maps = _host_inputs(x)
    res = bass_utils.run_bass_kernel_spmd(
        nc, in_maps, core_ids=list(range(N_CORES)))

    total_h = 0.0
    total_e = 0.0
    for c in range(N_CORES):
        o = res.results[c]["out"].astype(np.float64)
        total_h += o[:, 0].sum()
        total_e += o[:, 1].sum()

    # reference: 2 * (0.5 * sum_same dist) / (N * (BK - 1))
    #            2 * sum_diff relu(1 - dist) / (N * (N // BK - 1))
    # device accumulates sum(min(dist-1, 0) * mask_diff) = -heter partial
    homo = total_h / (N * (BK - 1))
    heter = -2.0 * total_e / (N * (N // BK - 1))
    return (np.float32(homo), np.float32(heter))


# revision 26
# speedup vs baseline: 1.5407x; 1.3178x over previous
"""Trainium2 Bass kernel for nn_MetricLoss (pairwise-distance metric loss).

Computation (reference):
    f = x.reshape(1024, 49152)
    G = f @ f.T                      (103 GFLOP Gram matrix)
    dist = sq_i + sq_j - 2 G         (the relu(dist) only binds on the
                                      diagonal, which both masks zero out)
    loss_homo  = 0.5 * sum(same-group dist)
    loss_heter = sum(cross-group relu(1 - dist))

Distribution (8 NeuronCores, one TRN2 chip):
    K-parallel with symmetry: core c holds f[:, c*6144:(c+1)*6144].T as a
    [48, 128, 1024] bf16 tensor (k-major tiles, fully SBUF-resident). Since
    the Gram matrix and both losses are symmetric, each core computes only
    the upper-triangle 128x128 blocks of its partial Gram (36/64 of the
    matmul work) via PSUM-accumulated chains over 4 column chunks, and the
    cross-block loss terms are counted once with weight 2 via a per-core
    weighted mask — the lower triangle is never materialized. Chunked bf16
    ReduceScatters give core c the upper-triangle part of full-K Gram rows
    [128c:128c+128]. Row norms sq are computed in fp32 on the otherwise-idle
    Scalar + Vector + GpSimd engines and summed across cores with a tiny
    fp32 AllReduce issued first (it also absorbs the collective-engine cold
    start). A fused DVE epilogue computes the masked hinge sums per chunk as
    each ReduceScatter lands; the host sums 8x[128,2] partials and
    normalizes.
"""

import numpy as np
import ml_dtypes

import concourse.bass as bass
import concourse.bacc as bacc
import concourse.tile as tile
import concourse.mybir as mybir
import concourse.bass_isa as bass_isa
from concourse.tile_rust import add_dep_helper
from concourse import bass_utils

F32 = mybir.dt.float32
BF16 = mybir.dt.bfloat16
ALU = mybir.AluOpType
AF = mybir.ActivationFunctionType

N_CORES = 8
N = 1024            # batch (rows of f)
K = 64 * 768        # 49152 features per sample
KC = K // N_CORES   # 6144 features per core
KT = KC // 128      # 48 k-tiles of 128 per core
BK = 8              # samples per class group
MB = N // 128       # 8 row blocks

CWS = [384, 256, 256, 128]          # column-chunk widths
OFF = [0, 384, 640, 896]            # chunk column offsets
NJ = len(CWS)
# process biggest chunks first (hides the input DMA), smallest last
CH_ORDER = [2, 1, 0, 3]

_CACHE = {}


def _chunk_chains(j):
    """Upper-triangle chains for chunk j: (m, col_off, width)."""
    chains = []
    for m in range(MB):
        lo = max(OFF[j], 128 * m)
        hi = OFF[j] + CWS[j]
        if lo < hi:
            chains.append((m, lo, hi - lo))
    return chains


def _build_nc():
    nc = bacc.Bacc("TRN2", target_bir_lowering=False, debug=False,
                   num_devices=N_CORES)

    ft = nc.dram_tensor("ft", [KT, 128, N], BF16, kind="ExternalInput").ap()
    mask_same = nc.dram_tensor("mask_same", [128, N], F32,
                               kind="ExternalInput").ap()
    # weighted cross-group mask: 0 below own block, 1 within own block,
    # 2 above it (upper-triangle pair counting)
    mask_diff = nc.dram_tensor("mask_diff", [128, N], F32,
                               kind="ExternalInput").ap()
    emask = nc.dram_tensor("emask", [128, 8], F32, kind="ExternalInput").ap()
    out = nc.dram_tensor("out", [128, 2], F32, kind="ExternalOutput").ap()

    rg = [list(range(N_CORES))]

    with tile.TileContext(nc) as tc:
        with (
            tc.tile_pool(name="ftp", bufs=1) as ftp,
            tc.tile_pool(name="misc", bufs=1) as misc,
            tc.tile_pool(name="gcopy", bufs=4) as gcp,
            tc.tile_pool(name="sqt", bufs=3) as sqtp,
            tc.tile_pool(name="junk", bufs=1) as jkp,
            tc.tile_pool(name="psum", bufs=8, space="PSUM") as psp,
            tc.tile_pool(name="dram", bufs=1, space="DRAM") as drp,
        ):
            # ---- load inputs to SBUF ----
            ft_sb = []
            for k in range(KT):
                t = ftp.tile([128, N], BF16, tag=f"ft{k}", name=f"ft{k}")
                nc.sync.dma_start(t[:], ft[k])
                ft_sb.append(t)

            ms_sb = misc.tile([128, N], F32, tag="ms", name="ms")
            md_sb = misc.tile([128, N], F32, tag="md", name="md")
            em_sb = misc.tile([128, 8], F32, tag="em", name="em")
            nc.sync.dma_start(ms_sb[:], mask_same[:])
            nc.sync.dma_start(md_sb[:], mask_diff[:])
            nc.sync.dma_start(em_sb[:], emask[:])

            # ---- sq pipeline on ACT (square) + DVE (accumulate) ----
            acc = misc.tile([128, N], F32, tag="acc", name="acc")
            nc.vector.memset(acc[:], 0.0)
            for k in range(KT):
                sqt = sqtp.tile([128, N], F32, tag="sqt", name=f"sqt{k}")
                nc.scalar.activation(sqt[:], ft_sb[k][:], AF.Square)
                nc.vector.tensor_tensor(acc[:], acc[:], sqt[:], ALU.add)

            sqb = drp.tile([1, N], F32, tag="sqb", name="sqb")
            sq_ar = drp.tile([1, N], F32, tag="sqar", name="sq_ar")
            par = misc.tile([128, N], F32, tag="par", name="par")
            nc.gpsimd.partition_all_reduce(
                par[:], acc[:], channels=128,
                reduce_op=bass_isa.ReduceOp.add)
            nc.sync.dma_start(sqb[:], par[0:1, :])

            bounce = {}
            rs = {}
            for j in range(NJ):
                bounce[j] = drp.tile([N, CWS[j]], BF16, tag=f"bnc{j}",
                                     name=f"bnc{j}")
                rs[j] = drp.tile([128, CWS[j]], BF16, tag=f"rs{j}",
                                 name=f"rs{j}")

            # zero the never-written (lower-triangle) bounce regions so the
            # ReduceScatter output is finite everywhere
            zero_sb = misc.tile([128, 384], BF16, tag="z", name="zero_sb")
            nc.vector.memset(zero_sb[:], 0.0)
            for j in range(NJ):
                nb = (OFF[j] + CWS[j]) // 128
                for m in range(nb, MB):
                    nc.sync.dma_start(
                        bounce[j][m * 128:(m + 1) * 128, :],
                        zero_sb[:, 0:CWS[j]])
                for (m, lo, w) in _chunk_chains(j):
                    if lo > OFF[j]:
                        nc.sync.dma_start(
                            bounce[j][m * 128:(m + 1) * 128, 0:lo - OFF[j]],
                            zero_sb[:, 0:lo - OFF[j]])

            # ---- upper-triangle partial Gram ----
            # collectives emitted + pinned in order:
            # RS[c2], RS[c1], AR_sq, RS[c0], RS[c3]
            prev_cc = None
            for idx, j in enumerate(CH_ORDER):
                chs = _chunk_chains(j)
                ptiles = {}
                for (m, lo, w) in chs:
                    ptiles[m] = psp.tile([128, w], F32, tag="chain",
                                         name=f"ch{j}_{m}")
                for k in range(KT):
                    for (m, lo, w) in chs:
                        nc.tensor.matmul(
                            ptiles[m][:],
                            lhsT=ft_sb[k][:, m * 128:(m + 1) * 128],
                            rhs=ft_sb[k][:, lo:lo + w],
                            start=(k == 0),
                            stop=(k == KT - 1),
                        )
                for (m, lo, w) in chs:
                    g = gcp.tile([128, w], BF16, tag="g", name=f"g{j}_{m}")
                    nc.vector.tensor_copy(g[:], ptiles[m][:])
                    nc.sync.dma_start(
                        bounce[j][m * 128:(m + 1) * 128,
                                  lo - OFF[j]:lo - OFF[j] + w], g[:])
                if idx == 2:
                    # sq AllReduce slotted mid-chain (its input is ready
                    # by then; results feed the per-chunk epilogues)
                    ar_cc = nc.gpsimd.collective_compute(
                        "AllReduce", ALU.add, replica_groups=rg,
                        ins=[sqb.opt()], outs=[sq_ar.opt()])
                    prev_cc = ar_cc
                rs_cc = nc.gpsimd.collective_compute(
                    "ReduceScatter", ALU.add, replica_groups=rg,
                    ins=[bounce[j].opt()], outs=[rs[j].opt()])
                prev_cc = rs_cc

            # ---- sq_row + sq_col broadcast ----
            flat_sb = misc.tile([1, N], F32, tag="flat", name="flat")
            nc.sync.dma_start(flat_sb[:], sq_ar[:])
            S_all = misc.tile([128, 8], F32, tag="S", name="S")
            for b in range(MB):
                nc.sync.dma_start(S_all[:, b:b + 1],
                                  sq_ar[0:1, b * 128:(b + 1) * 128])
            sq_row = misc.tile([128, 1], F32, tag="sqr", name="sqr")
            junk8 = misc.tile([128, 8], F32, tag="jk8", name="junk8")
            nc.vector.tensor_tensor(junk8[:], S_all[:], em_sb[:], ALU.mult)
            nc.vector.reduce_sum(sq_row[:], junk8[:],
                                 axis=mybir.AxisListType.X)
            ones = misc.tile([1, 128], F32, tag="ones", name="ones")
            nc.vector.memset(ones[:], 1.0)
            # PE is idle after the passes; PSUM chain slots are free
            Bs = []
            for h in range(2):
                Bh = psp.tile([128, 512], F32, tag="chain", name=f"B{h}")
                nc.tensor.matmul(Bh[:], lhsT=ones[:],
                                 rhs=flat_sb[0:1, h * 512:(h + 1) * 512],
                                 start=True, stop=True)
                Bs.append(Bh)

            # ---- per-chunk epilogue (fires as each RS lands) ----
            G_sb = misc.tile([128, N], BF16, tag="G", name="G")
            acc_h = []
            acc_e = []
            for j in CH_ORDER:
                W = CWS[j]
                nc.sync.dma_start(G_sb[:, OFF[j]:OFF[j] + W], rs[j][:, :])
                # split at the 512 boundary so each piece maps into one
                # B half
                pieces = []
                lo = OFF[j]
                hi = OFF[j] + W
                while lo < hi:
                    p_hi = min(hi, (lo // 512 + 1) * 512)
                    pieces.append((lo, p_hi - lo))
                    lo = p_hi
                for (plo, pw) in pieces:
                    sl = slice(plo, plo + pw)
                    t0 = jkp.tile([128, pw], F32, tag=f"t0_{plo}",
                                  name=f"t0_{plo}")
                    nc.vector.tensor_scalar_mul(t0[:], G_sb[:, sl], -2.0)
                    d = jkp.tile([128, pw], F32, tag=f"d{plo}",
                                 name=f"d{plo}")
                    Bsl = Bs[plo // 512][:, plo % 512:plo % 512 + pw]
                    nc.vector.scalar_tensor_tensor(
                        out=d[:], in0=t0[:], scalar=sq_row[:], in1=Bsl,
                        op0=ALU.add, op1=ALU.add)
                    ah = misc.tile([128, 1], F32, tag=f"ah{plo}",
                                   name=f"ah{plo}")
                    jh = jkp.tile([128, pw], F32, tag=f"jh{plo}",
                                  name=f"jh{plo}")
                    nc.vector.tensor_tensor(jh[:], d[:], ms_sb[:, sl],
                                            ALU.mult)
                    nc.vector.reduce_sum(ah[:], jh[:],
                                         axis=mybir.AxisListType.X)
                    acc_h.append(ah)
                    # min(d-1, 0) = -relu(1-d); heter partial =
                    # -sum(mask * that), negation applied on the host
                    t1 = jkp.tile([128, pw], F32, tag=f"t1_{plo}",
                                  name=f"t1_{plo}")
                    nc.vector.tensor_scalar(
                        t1[:], d[:], -1.0, 0.0, ALU.add, ALU.min)
                    eh = misc.tile([128, 1], F32, tag=f"eh{plo}",
                                   name=f"eh{plo}")
                    je = jkp.tile([128, pw], F32, tag=f"je{plo}",
                                  name=f"je{plo}")
                    nc.vector.tensor_tensor(je[:], t1[:], md_sb[:, sl],
                                            ALU.mult)
                    nc.vector.reduce_sum(eh[:], je[:],
                                         axis=mybir.AxisListType.X)
                    acc_e.append(eh)

            out_sb = misc.tile([128, 2], F32, tag="osb", name="osb")
            hsum = misc.tile([128, 2], F32, tag="hsum", name="hsum")
            esum = misc.tile([128, 2], F32, tag="esum", name="esum")
            nc.vector.tensor_tensor(hsum[:, 0:1], acc_h[0][:], acc_h[1][:],
                                    ALU.add)
            nc.vector.tensor_tensor(esum[:, 0:1], acc_e[0][:], acc_e[1][:],
                                    ALU.add)
            for i in range(2, len(acc_h) - 1):
                nc.vector.tensor_tensor(hsum[:, 0:1], hsum[:, 0:1],
                                        acc_h[i][:], ALU.add)
                nc.vector.tensor_tensor(esum[:, 0:1], esum[:, 0:1],
                                        acc_e[i][:], ALU.add)
            nc.vector.tensor_tensor(out_sb[:, 0:1], hsum[:, 0:1],
                                    acc_h[-1][:], ALU.add)
            nc.vector.tensor_tensor(out_sb[:, 1:2], esum[:, 0:1],
                                    acc_e[-1][:], ALU.add)
            nc.sync.dma_start(out[:], out_sb[:])

    nc.compile()
    return nc


def _host_inputs(x: np.ndarray):
    """Shard + transpose + cast x into per-core input maps."""
    f = np.ascontiguousarray(x.reshape(N, K))
    groups = np.arange(N) // BK
    cols = np.arange(N)

    in_maps = []
    for c in range(N_CORES):
        ftc = np.ascontiguousarray(
            f[:, c * KC:(c + 1) * KC].T).astype(ml_dtypes.bfloat16)
        rows = c * 128 + np.arange(128)
        g_r = groups[rows]
        same = ((g_r[:, None] == groups[None, :]) &
                (rows[:, None] != cols[None, :])).astype(np.float32)
        diff = (g_r[:, None] != groups[None, :]).astype(np.float32)
        # upper-triangle pair weighting: drop below-block pairs, count
        # above-block pairs twice (dist is symmetric)
        w = np.where(cols < 128 * c, 0.0,
                     np.where(cols < 128 * (c + 1), 1.0, 2.0))
        diff = diff * w[None, :].astype(np.float32)
        em = np.zeros((128, 8), dtype=np.float32)
        em[:, c] = 1.0
        in_maps.append({
            "ft": ftc.reshape(KT, 128, N),
            "mask_same": same,
            "mask_diff": diff,
            "emask": em,
        })
    return in_maps


def kernel(x: np.ndarray):
    if "nc" not in _CACHE:
        _CACHE["nc"] = _build_nc()
    nc = _CACHE["nc"]

    in_maps = _host_inputs(x)
    res = bass_utils.run_bass_kernel_spmd(
        nc, in_maps, core_ids=list(range(N_CORES)))

    total_h = 0.0
    total_e = 0.0
    for c in range(N_CORES):
        o = res.results[c]["out"].astype(np.float64)
        total_h += o[:, 0].sum()
        total_e += o[:, 1].sum()

    # reference: 2 * (0.5 * sum_same dist) / (N * (BK - 1))
    #            2 * sum_diff relu(1 - dist) / (N * (N // BK - 1))
    # device accumulates sum(min(dist-1, 0) * mask_diff) = -heter partial
    homo = total_h / (N * (BK - 1))
    heter = -2.0 * total_e / (N * (N // BK - 1))
    return (np.float32(homo), np.float32(heter))
